# revision 9
# baseline (speedup 1.0000x reference)
"""Corr2D (FlowNet-style correlation) Trainium2 Bass kernel.

Problem (hardcoded): x0, x1: [4, 64, 256, 512] f32.
  MAX_D=32, PAD=1, K=3, strides 1  ->  out [4, 65, 256, 512] f32
  out[b,d,h,w] = (1/576) * sum_{i,j in 0..2} sum_c x0p[b,c,h+i,w+j] * x1p[b,c,h+i,w+j+d]
  (x0p spatially zero-padded by 1; x1p width additionally padded by 32 each side)

Strategy (v3 — compound pair matmuls):
  - 8 cores = (batch b in 0..3) x (height half hh in 0..1). No communication.
  - Per padded row r and 128-wide w-tile the device computes the channel
    contraction band R_r[p, j] = sum_c x0p[c, r, u0+p] * x1p[c, r, u0+j]
    (the needed output is the diagonal band R[p, p+d], d=0..64). Both the
    3-row (h) fold and 3-col (w) fold happen on the host during unshard.
  - The kernel is PE-instruction-overhead bound (LDWEIGHTS is ~127ns flat
    regardless of shape; a matmul streams N cols at ~0.42ns each), so rows
    are processed in PAIRS with ONE weight load: x0 weights are loaded
    pair-stacked (partitions 0..63 = even row, 64..127 = odd row, K=128)
    and ONE compound matmul streams TWO rhs windows [128, 2, 192]:
      stream 0: x1 even row in partitions 0..63, zeros in 64..127 -> R_{2k}
      stream 1: zeros in 0..63, x1 odd row in 64..127            -> R_{2k+1}
    This costs 1 LDWEIGHTS + 384 stream cols per pair instead of 2+384.
    The zero partition-halves live in a persistent ring of 4 x1 tiles whose
    pad regions are memset once; chunk loads only touch the data regions.
  - Bands are copied PSUM->SBUF (f32->bf16) on a vector/scalar/gpsimd
    rotation and DMA'd as full [128, 26*192] blocks to a DRAM scratch with
    row pitch 26*192+1; the +1 lets a zero-copy numpy as_strided view read
    the diagonals on the host.
"""

import numpy as np

import concourse.bass as bass  # noqa: F401  (AP helpers)
import concourse.mybir as mybir
import concourse.tile as tile
from concourse import bacc
from concourse.bass_utils import run_bass_kernel_spmd

# ---- problem constants (hardcoded per contract) ----
B, C, H, W = 4, 64, 256, 512
ND = 65          # displacements 0..64 (= -32..32)
NROWS = 130      # local padded rows per core (128 out rows + 2)
HOUT = 128       # output rows per core
NWT = 4          # w tiles, bases U = 1 + 128*wt  (x0p col coords)
W0P = W + 2      # 514 x0p padded width
W1P = W + 66     # 578 x1p padded width
N_CORES = 8

# ---- layout tunables ----
MMN = 192              # band width per product (matmul N)
KH = 26                # row-products batched per band write DMA (13 pairs)
NBATCH = NROWS // KH   # 5 (130 = 5*26 exactly)
PITCH = KH * MMN + 1   # scratch row pitch; +1 gives the host-side shear
NPAIR = NROWS // 2     # 65 row pairs

_nc_cache = []


def _build_nc():
    """Build the per-core bass program."""
    nc = bacc.Bacc(None, target_bir_lowering=False)
    x0 = nc.dram_tensor("x0p", [C, NROWS, W0P], mybir.dt.bfloat16, kind="ExternalInput")
    x1 = nc.dram_tensor("x1p", [C, NROWS, W1P], mybir.dt.bfloat16, kind="ExternalInput")
    out = nc.dram_tensor(
        "scratch",
        [NBATCH * NWT, 128, PITCH],
        mybir.dt.bfloat16,
        kind="ExternalOutput",
    )

    x0v = x0.rearrange("c (r two) w -> c r two w", two=2)  # [C, 65, 2, W0P]
    x1v = x1.rearrange("c (r two) w -> c r two w", two=2)
    n_chunks = (NPAIR + 3) // 4  # 17 chunks of up to 4 pairs

    with tile.TileContext(nc) as tc:
        with (
            tc.tile_pool(name="x0pool", bufs=4) as p0,
            tc.tile_pool(name="x1pool", bufs=4) as p1,
            tc.tile_pool(name="spool", bufs=2) as ps,
            tc.tile_pool(name="psum", bufs=8, space="PSUM") as pp,
        ):
            x0c: dict[int, bass.AP] = {}

            # persistent ring of 4 x1 tiles; partition-half pad regions are
            # zeroed ONCE here and never touched by the chunk loads.
            x1ring = []
            for j in range(4):
                t = p1.tile(
                    [128, 4, 2, W1P], mybir.dt.bfloat16,
                    tag="x1ring", name=f"x1ring_{j}",
                )
                nc.gpsimd.memset(t[C : 2 * C, :, 0, :], 0)
                nc.gpsimd.memset(t[0:C, :, 1, :], 0)
                x1ring.append(t)

            def load_chunk(ci):
                q0 = 4 * ci
                pairs = min(4, NPAIR - q0)
                x0t = p0.tile([128, pairs, W0P], mybir.dt.bfloat16, tag="x0c")
                nc.gpsimd.dma_start(
                    out=x0t[0:C], in_=x0v[:, q0 : q0 + pairs, 0, :]
                )
                nc.gpsimd.dma_start(
                    out=x0t[C : 2 * C], in_=x0v[:, q0 : q0 + pairs, 1, :]
                )
                x0c[ci] = x0t
                xt = x1ring[ci % 4]
                nc.sync.dma_start(
                    out=xt[0:C, 0:pairs, 0, :], in_=x1v[:, q0 : q0 + pairs, 0, :]
                )
                nc.sync.dma_start(
                    out=xt[C : 2 * C, 0:pairs, 1, :],
                    in_=x1v[:, q0 : q0 + pairs, 1, :],
                )

            load_chunk(0)
            load_chunk(1)
            load_chunk(2)
            ncopy = 0
            for hb in range(NBATCH):
                sbufs = {}
                for wt in range(NWT):
                    sbufs[wt] = ps.tile(
                        [128, KH, MMN],
                        mybir.dt.bfloat16,
                        tag=f"s8_{wt}",
                        name=f"s8_{hb}_{wt}",
                    )
                for pl in range(KH // 2):  # 13 pairs per batch
                    p = hb * (KH // 2) + pl
                    ci, e = divmod(p, 4)
                    if e == 0 and ci + 2 < n_chunks and ci + 2 not in x0c:
                        load_chunk(ci + 2)
                    for wt in range(NWT):
                        ug = 1 + 128 * wt
                        pt = pp.tile([128, 2, MMN], mybir.dt.float32, tag="pt")
                        nc.tensor.matmul(
                            out=pt,
                            lhsT=x0c[ci][:, e, ug : ug + 128],
                            rhs=x1ring[ci % 4][:, e, :, ug : ug + MMN],
                            start=True,
                            stop=True,
                        )
                        dst = sbufs[wt][:, 2 * pl : 2 * pl + 2, :]
                        ncopy += 1
                        if ncopy % 2 == 0:
                            nc.vector.tensor_copy(out=dst, in_=pt)
                        else:
                            nc.scalar.copy(out=dst, in_=pt)
                # full band [128, KH*MMN] per wt, written with the pitch shear
                for wt in range(NWT):
                    blk = hb * NWT + wt
                    eng = nc.sync if (wt % 2 == 0) else nc.scalar
                    eng.dma_start(
                        out=out[blk, :, 0 : KH * MMN],
                        in_=sbufs[wt][:, :, :],
                    )
    nc.finalize()
    return nc


def _get_nc():
    if not _nc_cache:
        _nc_cache.append(_build_nc())
    return _nc_cache[0]


def _core_inputs(x0, x1, core):
    b, hh = divmod(core, 2)
    zrow = np.zeros((C, 1, W), np.float32)
    if hh == 0:
        s0 = np.concatenate([zrow, x0[b, :, 0 : HOUT + 1, :]], axis=1)
        s1 = np.concatenate([zrow, x1[b, :, 0 : HOUT + 1, :]], axis=1)
    else:
        s0 = np.concatenate([x0[b, :, HOUT - 1 : H, :], zrow], axis=1)
        s1 = np.concatenate([x1[b, :, HOUT - 1 : H, :], zrow], axis=1)
    import ml_dtypes

    x0p = np.zeros((C, NROWS, W0P), ml_dtypes.bfloat16)
    x0p[:, :, 1 : 1 + W] = s0.astype(ml_dtypes.bfloat16)
    x1p = np.zeros((C, NROWS, W1P), ml_dtypes.bfloat16)
    x1p[:, :, 33 : 33 + W] = s1.astype(ml_dtypes.bfloat16)
    return {"x0p": np.ascontiguousarray(x0p), "x1p": np.ascontiguousarray(x1p)}


def _unshard(results, esz=2):
    out = np.empty((B, ND, H, W), np.float32)
    for core in range(N_CORES):
        s = np.ascontiguousarray(results[core]["scratch"])
        flat = s.reshape(-1)
        # V[hb, wt, a, k, d] = flat[(hb*NWT+wt)*128*PITCH
        #                           + a*(PITCH+1) + k*MMN + d]
        v = np.lib.stride_tricks.as_strided(
            flat,
            shape=(NBATCH, NWT, 128, KH, ND),
            strides=(
                NWT * 128 * PITCH * esz,
                128 * PITCH * esz,
                (PITCH + 1) * esz,
                MMN * esz,
                esz,
            ),
        )
        vf = v.astype(np.float32)
        # -> [d, (hb,k)=r, (wt,a)=w]
        pd = np.ascontiguousarray(vf.transpose(4, 0, 3, 1, 2)).reshape(
            ND, NROWS, W
        )
        ph = pd[:, 0:HOUT] + pd[:, 1 : HOUT + 1] + pd[:, 2 : HOUT + 2]
        oh = ph.copy()
        oh[:, :, 1:] += ph[:, :, :-1]
        oh[:, :, :-1] += ph[:, :, 1:]
        oh *= 1.0 / 576.0
        b, hh = divmod(core, 2)
        out[b, :, hh * HOUT : (hh + 1) * HOUT, :] = oh
    return out


def kernel(x0, x1, trace=False):
    x0 = np.asarray(x0, dtype=np.float32)
    x1 = np.asarray(x1, dtype=np.float32)
    nc = _get_nc()
    in_maps = [_core_inputs(x0, x1, core) for core in range(N_CORES)]
    res = run_bass_kernel_spmd(nc, in_maps, core_ids=list(range(N_CORES)), trace=trace)
    out = _unshard(res.results)
    if trace:
        kernel.last_result = res
    return out
